# revision 1
# baseline (speedup 1.0000x reference)
"""AttnBlock (GroupNorm + single-head self-attention + proj + residual) on 8 trn2 cores.

Sharding: core = (batch b = core//4, query-block qb = core%4). Each core gets its
batch's x rolled so its 1024 queries are columns 0:1024; attention key/value
order is permutation-invariant so the roll is free. No cross-core communication.

Math (validated in numpy to 7e-8 rel err): the four 1x1 convs are fused on the
HOST into two C*C matrices (pure weight preprocessing, no data dependence):
    K2  = wq^T @ wk          (logits bilinear form:  l[i,j] = hn_i^T K2 hn_j)
    W3  = wo @ wv            (value+proj fused)
GroupNorm folds into a per-channel affine hn = A*x + B on device:
    qk2[b,i] = A[b] * (sum_a A[a] K2[a,b] x[a,i]) + A[b]*cb[b],
      cb = K2^T B + wk^T bq;  pure-B logits terms are constant per query and
      cancel in softmax, exactly like the k-bias.
    logitsT[j,i] = sum_b x[b,j] qk2[b,i]   (keys-major, no transposes)
    P = exp(logitsT/sqrt(C)) unnormalized
    o = W3A @ (x @ P)  <- keys contracted FIRST (4096 -> the C*C proj only
        touches the 1024-query result), using a keys-major x^T built by PE
        transposes; out = o/s + fb + x with fb = W3T^T B + wo@bv + bo and
        s = column sums of P.

Schedule notes (perfetto-driven):
  - DMA order on the single sync-queue FIFO: vp/selT (tiny), K2, x in 10
    pieces, W3T. Bulk stream starts ~9us after the fixed framework preamble;
    x fully lands ~31us at ~420 GB/s.
  - HAM clock gate: 2.4 GHz needs ~3.4us of SUSTAINED PE busy; a 16-matmul
    512-wide burst pinned to a late x piece warms it, per-piece singles and
    a small pacer block over the stats chain hold it until the matmul stream
    becomes continuous.
  - Stats: bn_stats per 512-col chunk as each DMA piece lands; group reduce
    via a tiny select matmul; batched ACT sqrts (one table load), then the
    A,B affine; A is folded into K2/W3T rows in place.
  - x^T is built by PE transpose instructions in the DMA shadow (they also
    keep the clock gate warm); an Exp-table preload rides after the sqrts.
  - Main loop per key tile (both chunks): 4 logits mms, exp on ACT, 4 x@P
    accumulation mms lagged one iteration so the exp latency is hidden.
    P row-sums accumulate on DVE (jt<=30; the last tile joins via a second
    accumulating ones-matmul so 1/s is ready early).
  - Chunk end: ACT drains x@P to SBUF as each bank stops, 16 small matmuls
    apply W3A, DVE normalizes by 1/s (freeing PSUM banks for the next chunk)
    and adds xq = x + fb; DMA out per 128-row block.
"""

import numpy as np

import concourse.bass as bass
import concourse.bacc as bacc
import concourse.tile as tile
from concourse import mybir
from concourse.bass_utils import run_bass_kernel_spmd

F32 = mybir.dt.float32
F32R = mybir.dt.float32r
AF = mybir.ActivationFunctionType
ALU = mybir.AluOpType
AX = mybir.AxisListType

B, C, HH, WW = 2, 512, 64, 64
N = HH * WW          # 4096 pixels
NQ = N // 4          # queries per core
G = 32               # groups
GPT = 8              # groups per 128-channel tile
NT = C // 128        # 4 channel tiles
JT = N // 128        # 32 key tiles
CW = 512             # query chunk width
NCH = NQ // CW       # 2 chunks per core
EPS = 1e-6
SCALE = float(C) ** -0.5
GDIV = 1.0 / 16.0    # 16 channels per group

_CACHE: dict = {}


def _f32(ap):
    return ap.bitcast(F32)


def _build_bass():
    nc = bacc.Bacc("TRN2")

    x_d = nc.declare_dram_parameter("x", [C, N], F32R, isOutput=False)
    k2_d = nc.declare_dram_parameter("K2", [C, C], F32R, isOutput=False)
    w3_d = nc.declare_dram_parameter("W3T", [C, C], F32R, isOutput=False)
    vp_d = nc.declare_dram_parameter("vp", [128, 24], F32, isOutput=False)
    selT_d = nc.declare_dram_parameter("selT", [GPT, 128], F32, isOutput=False)
    id_d = nc.declare_dram_parameter("ident", [128, 128], F32, isOutput=False)
    out_d = nc.declare_dram_parameter("out", [C, NQ], F32, isOutput=True)

    dram = dict(x=x_d, K2=k2_d, W3T=w3_d, vp=vp_d, selT=selT_d, ident=id_d,
                out=out_d)
    with tile.TileContext(nc) as tc, \
         nc.allow_low_precision(reason="float32r tiles are 4-byte fp32 feeding the PE"):
        _emit(tc, {k: v.ap() for k, v in dram.items()})
    nc.compile()
    return nc


def _emit(tc, d):
    nc = tc.nc

    # ---- long-lived pools -------------------------------------------------
    xp = tc.alloc_tile_pool(name="xp", bufs=NT)
    k2p = tc.alloc_tile_pool(name="k2p", bufs=NT)
    w3p = tc.alloc_tile_pool(name="w3p", bufs=NT)
    vecs = tc.alloc_tile_pool(name="vecs", bufs=1)
    xtp = tc.alloc_tile_pool(name="xtp", bufs=1)
    xps = tc.alloc_tile_pool(name="xps", bufs=NT)
    xqp = tc.alloc_tile_pool(name="xqp", bufs=NT)

    # ones tile via memset: warm-up lhsT + softmax-sum matmuls, no DMA.
    ones32_sb = vecs.tile([128, 128], F32, tag="ones32")
    nc.vector.memset(ones32_sb[:, :], 1.0)
    ones128_sb = vecs.tile([128, 128], F32R, tag="ones128")
    nc.vector.tensor_copy(out=ones128_sb[:, :], in_=ones32_sb[:, :])

    # ---- DMA in (sync-queue FIFO: tiny first, then K2, x, W3T) ------------
    vp_sb = vecs.tile([128, 24], F32, tag="vp")
    nc.sync.dma_start(out=vp_sb[:, :], in_=d["vp"])
    selT_sb = vecs.tile([GPT, 128], F32, tag="selT")
    nc.sync.dma_start(out=selT_sb[:, :], in_=d["selT"])
    id_sb = vecs.tile([128, 128], F32, tag="ident")
    nc.sync.dma_start(out=id_sb[:, :], in_=d["ident"])

    gnw_sb = vp_sb[:, 0:NT]
    gnb_sb = vp_sb[:, NT:2 * NT]
    wkbq_sb = vp_sb[:, 2 * NT:3 * NT]
    wobv_sb = vp_sb[:, 3 * NT:4 * NT]
    sel_sb = vp_sb[:, 4 * NT:4 * NT + GPT]

    def load_w(pool, name, tag):
        tiles = []
        r = d[name].rearrange("(t p) m -> t p m", p=128)
        for t in range(NT):
            wt = pool.tile([128, C], F32R, tag=tag)
            nc.sync.dma_start(out=wt[:, :], in_=r[t])
            tiles.append(wt)
        return tiles

    k2_sb = load_w(k2p, "K2", "K2")    # [a_part, b] raw; A-scaled in place later

    x_sb = []
    x_t = d["x"].rearrange("(t p) n -> t p n", p=128)
    xsplits = []
    for t in range(NT):
        xt = xp.tile([128, N], F32R, tag="x", name=f"xt{t}")
        nsp = 2 if t < NT - 1 else 4   # last tile lands in quarters
        w = N // nsp
        for hh in range(nsp):
            nc.sync.dma_start(out=xt[:, hh * w:(hh + 1) * w],
                              in_=x_t[t][:, hh * w:(hh + 1) * w])
        xsplits.append([(hh * w, (hh + 1) * w) for hh in range(nsp)])
        x_sb.append(xt)

    w3_sb = load_w(w3p, "W3T", "W3T")  # [b_part, co] raw; A-scaled in place later

    A_sb = vecs.tile([128, NT], F32, tag="A")
    B_sb = vecs.tile([128, NT], F32, tag="B")
    cbA_sb = vecs.tile([128, NT], F32, tag="cbA")
    wkbqA_sb = vecs.tile([128, NT], F32, tag="wkbqA")
    fb_sb = vecs.tile([128, NT], F32, tag="fb")

    # keys-major x^T, built by PE transposes in the DMA shadow
    xT_sb = xtp.tile([128, JT * C], F32R, tag="xT")
    xTr = xT_sb.rearrange("p (j c) -> p j c", c=C)

    # ---- GroupNorm stats -> per-channel affine A, B -----------------------
    with tc.tile_pool(name="stp", bufs=4) as stp, \
         tc.tile_pool(name="psxt", bufs=2, space="PSUM") as ps_xt, \
         tc.tile_pool(name="pssm", bufs=2, space="PSUM") as ps_sm:
        nwarm = [0]

        def emit_warm(n, rhs=None):
            for _ in range(n):
                wt = ps_sm.tile([128, CW], F32, tag="warm", name=f"wm{nwarm[0]}")
                nwarm[0] += 1
                r = ones128_sb[:, :] if rhs is None else rhs
                nc.tensor.matmul(out=wt[:, 0:128] if rhs is None else wt[:, :],
                                 lhsT=ones128_sb[:, :],
                                 rhs=r, start=True, stop=True)

        gps_t = []
        for t in range(NT):
            st = stp.tile([128, 8, 6], F32, tag="bnst", name=f"bnst{t}")
            xr = _f32(x_sb[t][:, :]).rearrange("p (s n) -> p s n", s=8)
            for pi, (lo, hi) in enumerate(xsplits[t]):
                # transpose this piece's 128-col blocks into keys-major x^T;
                # real PE work pinned to every DMA landing keeps the HAM
                # clock gate warm through the whole prologue.
                for q0 in range(lo // 128, hi // 128, 4):
                    tp = ps_xt.tile([128, CW], F32, tag="xt",
                                    name=f"tp{t}_{q0}")
                    for k in range(4):
                        jq = q0 + k
                        nc.tensor.transpose(
                            out=tp[:, k * 128:(k + 1) * 128],
                            in_=_f32(x_sb[t][:, jq * 128:(jq + 1) * 128]),
                            identity=id_sb[:, :])
                    nc.scalar.activation(
                        out=xTr[:, q0:q0 + 4, t * 128:(t + 1) * 128],
                        in_=tp[:, :].rearrange("p (k c) -> p k c", c=128),
                        func=AF.Copy, bias=0.0, scale=1.0)
                for s in range(lo // 512, hi // 512):
                    nc.vector.bn_stats(out=st[:, s, :], in_=xr[:, s, :])
            mv = stp.tile([128, 2], F32, tag="mv", name=f"mv{t}")
            nc.vector.bn_aggr(out=mv[:, :], in_=st[:, :, :])
            st2 = stp.tile([128, 2], F32, tag="st2", name=f"st2_{t}")
            nc.vector.tensor_copy(out=st2[:, 0:1], in_=mv[:, 0:1])
            nc.vector.tensor_mul(out=st2[:, 1:2], in0=mv[:, 0:1], in1=mv[:, 0:1])
            nc.vector.tensor_add(out=st2[:, 1:2], in0=st2[:, 1:2], in1=mv[:, 1:2])
            gps = ps_sm.tile([GPT, 2], F32, tag="gps", name=f"gps{t}")
            nc.tensor.matmul(out=gps[:, :], lhsT=sel_sb, rhs=st2[:, :],
                             start=True, stop=True)
            gps_t.append(gps)
        # pacer: keep the clock gate warm across the stats->A chain (the
        # last x piece has landed; no more DMA-pinned singles can fire).
        # 512-wide so the block really covers ~5us of PE time.
        emit_warm(24, rhs=x_sb[0][:, 0:CW])

        # group mean / rstd; all DVE preps first, then batched ACT sqrts
        grp_t = []
        for t in range(NT):
            grp = stp.tile([GPT, 2], F32, tag="grp", name=f"grp{t}")
            nc.vector.tensor_scalar_mul(out=grp[:, :], in0=gps_t[t][:, :], scalar1=GDIV)
            gtmp = stp.tile([GPT, 1], F32, tag="gtmp", name=f"gtmp{t}")
            nc.vector.tensor_mul(out=gtmp[:, :], in0=grp[:, 0:1], in1=grp[:, 0:1])
            nc.vector.tensor_sub(out=grp[:, 1:2], in0=grp[:, 1:2], in1=gtmp[:, :])
            nc.vector.tensor_scalar_add(out=grp[:, 1:2], in0=grp[:, 1:2], scalar1=EPS)
            grp_t.append(grp)
        for t in range(NT):
            nc.scalar.activation(out=grp_t[t][:, 1:2], in_=grp_t[t][:, 1:2],
                                 func=AF.Sqrt, bias=0.0, scale=1.0)
        # dummy exp: pulls the ~2.7us Exp ACT_TABLE_LOAD (observed firing at
        # loop start, on the critical path) into the prologue shadow
        scr = stp.tile([128, 1], F32, tag="scr")
        nc.scalar.activation(out=scr[:, :], in_=ones32_sb[:, 0:1], func=AF.Exp,
                             bias=0.0, scale=1.0)
        emit_warm(10, rhs=x_sb[0][:, 0:CW])
        for t in range(NT):
            nc.vector.reciprocal(out=grp_t[t][:, 1:2], in_=grp_t[t][:, 1:2])
            mrp = ps_sm.tile([128, 2], F32, tag="gps", name=f"mrp{t}")
            nc.tensor.matmul(out=mrp[:, :], lhsT=selT_sb[:, :], rhs=grp_t[t][:, :],
                             start=True, stop=True)
            tcol = slice(t, t + 1)
            nc.vector.tensor_mul(out=A_sb[:, tcol], in0=gnw_sb[:, tcol], in1=mrp[:, 1:2])
            nc.vector.tensor_mul(out=B_sb[:, tcol], in0=mrp[:, 0:1], in1=A_sb[:, tcol])
            nc.vector.tensor_sub(out=B_sb[:, tcol], in0=gnb_sb[:, tcol], in1=B_sb[:, tcol])

    ps_mm = tc.alloc_tile_pool(name="psmm", bufs=3, space="PSUM")

    # ---- qk bias cb = K2^T B + wk^T bq (needs raw K2, so before scaling) --
    nc.vector.tensor_mul(out=wkbqA_sb[:, :], in0=A_sb[:, :], in1=wkbq_sb)
    for bb in range(NT):
        bps = ps_mm.tile([128, 1], F32, tag="mm", name=f"cb{bb}")
        for a in range(NT):
            nc.tensor.matmul(out=bps[:, :],
                             lhsT=_f32(k2_sb[a][:, bb * 128:(bb + 1) * 128]),
                             rhs=B_sb[:, a:a + 1],
                             start=(a == 0), stop=(a == NT - 1))
        # cbA = A*(cb_psum) + A*wkbq
        nc.vector.tensor_scalar(out=cbA_sb[:, bb:bb + 1], in0=bps[:, :],
                                scalar1=A_sb[:, bb:bb + 1],
                                scalar2=wkbqA_sb[:, bb:bb + 1],
                                op0=ALU.mult, op1=ALU.add)

    # ---- K2A = A (.) K2 in place, then qk2 chunk 0 ------------------------
    for a in range(NT):
        nc.vector.tensor_scalar_mul(out=k2_sb[a][:, :], in0=_f32(k2_sb[a][:, :]),
                                    scalar1=A_sb[:, a:a + 1])

    qkp = tc.alloc_tile_pool(name="qkp", bufs=NT)

    def emit_qk(ch):
        csl = slice(ch * CW, (ch + 1) * CW)
        qk2 = []
        for bb in range(NT):
            qps = ps_mm.tile([128, CW], F32, tag="mm")
            for a in range(NT):
                nc.tensor.matmul(out=qps[:, :],
                                 lhsT=k2_sb[a][:, bb * 128:(bb + 1) * 128],
                                 rhs=x_sb[a][:, csl],
                                 start=(a == 0), stop=(a == NT - 1))
            qk = qkp.tile([128, CW], F32R, tag="qk")
            nc.vector.tensor_scalar(out=qk[:, :], in0=qps[:, :],
                                    scalar1=A_sb[:, bb:bb + 1],
                                    scalar2=cbA_sb[:, bb:bb + 1],
                                    op0=ALU.mult, op1=ALU.add)
            qk2.append(qk)
        return qk2

    qk2_ch = emit_qk(0)

    # ---- out bias fb = W3T^T B + (wo@bv + bo) (raw W3T, before scaling) ---
    for cob in range(NT):
        fps = ps_mm.tile([128, 1], F32, tag="mm", name=f"fb{cob}")
        for b in range(NT):
            nc.tensor.matmul(out=fps[:, :],
                             lhsT=_f32(w3_sb[b][:, cob * 128:(cob + 1) * 128]),
                             rhs=B_sb[:, b:b + 1],
                             start=(b == 0), stop=(b == NT - 1))
        nc.vector.tensor_add(out=fb_sb[:, cob:cob + 1], in0=fps[:, :],
                             in1=wobv_sb[:, cob:cob + 1])

    # ---- W3AT = A (.) W3T in place ----------------------------------------
    for b in range(NT):
        nc.vector.tensor_scalar_mul(out=w3_sb[b][:, :], in0=_f32(w3_sb[b][:, :]),
                                    scalar1=A_sb[:, b:b + 1])

    for k in range(6):
        wt = ps_mm.tile([128, CW], F32, tag="mm", name=f"wmq{k}")
        nc.tensor.matmul(out=wt[:, :], lhsT=ones128_sb[:, :],
                         rhs=x_sb[0][:, 0:CW], start=True, stop=True)

    # xq = x[:, 0:NQ] + fb (DVE; GpSimd is ~10x slower and steals SBUF ports)
    xq_sb = []
    for co in range(NT):
        xq = xqp.tile([128, NQ], F32, tag="xq", name=f"xq{co}")
        for h in range(NCH):
            sl = slice(h * CW, (h + 1) * CW)
            nc.vector.tensor_scalar_add(out=xq[:, sl], in0=_f32(x_sb[co][:, sl]),
                                        scalar1=fb_sb[:, co:co + 1])
        xq_sb.append(xq)

    # ---- attention chunks -------------------------------------------------
    ps_o = tc.alloc_tile_pool(name="pso", bufs=NT, space="PSUM")
    pp = tc.alloc_tile_pool(name="pp", bufs=6)
    outp = tc.alloc_tile_pool(name="outp", bufs=2)
    smsb = tc.alloc_tile_pool(name="smsb", bufs=2)

    qk2_next = qk2_ch

    for ch in range(NCH):
        csl = slice(ch * CW, (ch + 1) * CW)
        qk2_ch = qk2_next

        o_ps = [ps_o.tile([128, CW], F32, tag="o", name=f"o{ch}_{i}") for i in range(4)]
        sacc = smsb.tile([128, CW], F32R, tag="sacc", name=f"sacc{ch}")
        P_t = [None] * JT
        for jt in range(JT):
            jsl = slice(jt * 128, (jt + 1) * 128)
            lps = ps_mm.tile([128, CW], F32, tag="mm")
            for b in range(NT):
                nc.tensor.matmul(out=lps[:, :], lhsT=x_sb[b][:, jsl],
                                 rhs=qk2_ch[b][:, :],
                                 start=(b == 0), stop=(b == NT - 1))
            P = pp.tile([128, CW], F32R, tag="P")
            nc.scalar.activation(out=P[:, :], in_=lps[:, :], func=AF.Exp,
                                 bias=0.0, scale=SCALE)
            P_t[jt] = P
            # xP = x @ P accumulation lags one iteration: P[jt-1] is ready
            if jt > 0:
                for b in range(4):
                    nc.tensor.matmul(out=o_ps[b][:, :],
                                     lhsT=xTr[:, jt - 1, b * 128:(b + 1) * 128],
                                     rhs=P_t[jt - 1][:, :],
                                     start=(jt == 1), stop=False,
                                     skip_group_check=True)
            # running softmax denominator on DVE (jt<=30; P31 via matmul)
            if jt == 0:
                nc.vector.tensor_copy(out=sacc[:, :], in_=_f32(P[:, :]))
            elif jt < JT - 1:
                nc.vector.tensor_add(out=sacc[:, :], in0=_f32(sacc[:, :]),
                                     in1=_f32(P[:, :]))

        # 1/s: s = ones@sacc + ones@P31, ready before the epilogue needs it;
        # each x@P bank drains to SBUF (ACT) right as its last matmul stops
        xP_sb = []
        for b in range(4):
            nc.tensor.matmul(out=o_ps[b][:, :],
                             lhsT=xTr[:, JT - 1, b * 128:(b + 1) * 128],
                             rhs=P_t[JT - 1][:, :],
                             start=False, stop=True, skip_group_check=True)
            xs = xps.tile([128, CW], F32R, tag="xps", name=f"xps{ch}_{b}")
            nc.scalar.activation(out=xs[:, :], in_=o_ps[b][:, :], func=AF.Copy,
                                 bias=0.0, scale=1.0)
            xP_sb.append(xs)
        rbp = ps_mm.tile([128, CW], F32, tag="mm")
        nc.tensor.matmul(out=rbp[:, :], lhsT=ones128_sb[:, :], rhs=sacc[:, :],
                         start=True, stop=False)
        nc.tensor.matmul(out=rbp[:, :], lhsT=ones128_sb[:, :],
                         rhs=P_t[JT - 1][:, :], start=False, stop=True)
        rsb = smsb.tile([128, CW], F32, tag="rsb")
        nc.vector.reciprocal_approx_fast(out=rsb[:, :], in_=rbp[:, :])
        if ch + 1 < NCH:
            qk2_next = emit_qk(ch + 1)

        # o = W3A @ xP (16 small matmuls), normalize, +x+fb, DMA out
        for co in range(4):
            ops = ps_mm.tile([128, CW], F32, tag="mm")
            for b in range(4):
                nc.tensor.matmul(out=ops[:, :],
                                 lhsT=w3_sb[b][:, co * 128:(co + 1) * 128],
                                 rhs=xP_sb[b][:, :],
                                 start=(b == 0), stop=(b == 3))
            ot_ = outp.tile([128, CW], F32, tag="osb", name=f"n{ch}_{co}")
            nc.vector.tensor_mul(out=ot_[:, :], in0=ops[:, :], in1=rsb[:, :])
            ou = outp.tile([128, CW], F32, tag="oadd", name=f"r{ch}_{co}")
            nc.vector.tensor_add(out=ou[:, :], in0=ot_[:, :],
                                 in1=xq_sb[co][:, csl])
            nc.sync.dma_start(out=d["out"][co * 128:(co + 1) * 128, csl], in_=ou[:, :])

    for p in (smsb, outp, pp, ps_o, qkp, ps_mm, xqp, xps, xtp, vecs, w3p, k2p, xp):
        p.release()


def _sel_consts():
    sel = np.zeros((128, GPT), np.float32)
    for p in range(128):
        sel[p, p // 16] = 1.0
    return sel, np.ascontiguousarray(sel.T)


def kernel(x, gn_w, gn_b, wq, bq, wk, bk, wv, bv, wo, bo):
    del bk  # exactly cancelled by softmax shift invariance
    if "nc" not in _CACHE:
        _CACHE["nc"] = _build_bass()
    nc = _CACHE["nc"]

    x = np.ascontiguousarray(np.asarray(x, np.float32)).reshape(B, C, N)
    wq64 = np.asarray(wq, np.float64)
    wk64 = np.asarray(wk, np.float64)
    wv64 = np.asarray(wv, np.float64)
    wo64 = np.asarray(wo, np.float64)
    K2 = np.ascontiguousarray((wq64.T @ wk64).astype(np.float32))
    W3T = np.ascontiguousarray((wo64 @ wv64).T.astype(np.float32))
    wkbq = (wk64.T @ np.asarray(bq, np.float64)).astype(np.float32)
    wobvbo = (wo64 @ np.asarray(bv, np.float64)
              + np.asarray(bo, np.float64)).astype(np.float32)
    sel, selT = _sel_consts()
    ident = np.eye(128, dtype=np.float32)
    vp = np.concatenate([
        np.asarray(gn_w, np.float32).reshape(NT, 128).T,
        np.asarray(gn_b, np.float32).reshape(NT, 128).T,
        wkbq.reshape(NT, 128).T,
        wobvbo.reshape(NT, 128).T,
        sel,
    ], axis=1)
    vp = np.ascontiguousarray(vp)

    in_maps = []
    for core in range(8):
        b, qb = core // 4, core % 4
        xb = np.ascontiguousarray(np.roll(x[b], -qb * NQ, axis=1))
        in_maps.append({"x": xb, "K2": K2, "W3T": W3T, "vp": vp, "selT": selT,
                        "ident": ident})

    _CACHE["last_in_maps"] = in_maps
    res = run_bass_kernel_spmd(nc, in_maps, list(range(8))).results
    out = np.empty((B, C, N), np.float32)
    for core in range(8):
        b, qb = core // 4, core % 4
        out[b][:, qb * NQ:(qb + 1) * NQ] = res[core]["out"]
    return out.reshape(B, C, HH, WW)



# revision 3
# speedup vs baseline: 1.1687x; 1.1687x over previous
"""AttnBlock (GroupNorm + single-head self-attention + proj + residual) on 8 trn2 cores.

Sharding: core = (batch b = core//4, query-block qb = core%4). Each core gets its
batch's x rolled so its 1024 queries are columns 0:1024; attention key/value
order is permutation-invariant so the roll is free. No cross-core communication.

Math (validated in numpy, bf16 pipeline sim rel err 1.7e-3): the four 1x1 convs
are fused on the HOST into two C*C matrices (pure weight preprocessing):
    K2  = wq^T @ wk          (logits bilinear form:  l[i,j] = hn_i^T K2 hn_j)
    W3  = wo @ wv            (value+proj fused)
GroupNorm folds into a per-channel affine hn = A*x + B on device:
    qk2[b,i] = A[b] * (sum_a A[a] K2[a,b] x[a,i]) + A[b]*cb[b],
      cb = K2^T B + wk^T bq;  pure-B logits terms are constant per query and
      cancel in softmax, exactly like the k-bias.
    logitsT[j,i] = sum_b x[b,j] qk2[b,i]   (keys-major, no transposes)
    P = exp(logitsT/sqrt(C)) unnormalized
    o = W3A @ (x @ P)  <- keys contracted FIRST; out = o/s + fb + x with
    fb = W3T^T B + wo@bv + bo and s = column sums of P.

v2 changes (trace-driven, baseline 210us):
  - bf16 datapath: x, xT, K2, W3T, qk2, P, xs tiles are bf16 (tolerance is
    2e-2; measured pipeline error ~1.7e-3). Halves input DMA (x lands ~15us
    earlier) and halves LDWEIGHTS time on the PE.
  - x^T is now precomputed on the HOST and DMA'd (streams during the main
    loop); removes 128 PE transposes + 32 ACT copies from the prologue.
  - Clock-gate pacing: self-timed warm matmuls from t=0 through the DMA
    stream (pinned to piece landings), bridging the stats chain, so the
    main loop starts at full clock without serial warmup blocks.
  - Epilogue PSUM drains split ACT/DVE (2 banks each).
"""

import numpy as np
import ml_dtypes

import concourse.bass as bass
import concourse.bacc as bacc
import concourse.tile as tile
from concourse import mybir
from concourse.bass_utils import run_bass_kernel_spmd

F32 = mybir.dt.float32
F32R = mybir.dt.float32r
BF16 = mybir.dt.bfloat16
AF = mybir.ActivationFunctionType
ALU = mybir.AluOpType
AX = mybir.AxisListType

B, C, HH, WW = 2, 512, 64, 64
N = HH * WW          # 4096 pixels
NQ = N // 4          # queries per core
G = 32               # groups
GPT = 8              # groups per 128-channel tile
NT = C // 128        # 4 channel tiles
JT = N // 128        # 32 key tiles
CW = 512             # query chunk width
NCH = NQ // CW       # 2 chunks per core
EPS = 1e-6
SCALE = float(C) ** -0.5
GDIV = 1.0 / 16.0    # 16 channels per group

_CACHE: dict = {}


def _f32(ap):
    return ap.bitcast(F32)


def _build_bass():
    nc = bacc.Bacc("TRN2")

    x_d = nc.declare_dram_parameter("x", [C, N], BF16, isOutput=False)
    xt_d = nc.declare_dram_parameter("xT", [128, JT * C], BF16, isOutput=False)
    k2_d = nc.declare_dram_parameter("K2", [C, C], BF16, isOutput=False)
    w3_d = nc.declare_dram_parameter("W3T", [C, C], BF16, isOutput=False)
    vp_d = nc.declare_dram_parameter("vp", [128, 24], F32, isOutput=False)
    selT_d = nc.declare_dram_parameter("selT", [GPT, 128], F32, isOutput=False)
    out_d = nc.declare_dram_parameter("out", [C, NQ], F32, isOutput=True)

    dram = dict(x=x_d, xT=xt_d, K2=k2_d, W3T=w3_d, vp=vp_d, selT=selT_d,
                out=out_d)
    with tile.TileContext(nc) as tc, \
         nc.allow_low_precision(reason="bf16 pipeline validated at 1.7e-3 rel err vs 2e-2 tol"):
        _emit(tc, {k: v.ap() for k, v in dram.items()})
    nc.compile()
    return nc


def _emit(tc, d):
    nc = tc.nc

    # ---- long-lived pools -------------------------------------------------
    xp = tc.alloc_tile_pool(name="xp", bufs=NT)
    k2p = tc.alloc_tile_pool(name="k2p", bufs=NT)
    w3p = tc.alloc_tile_pool(name="w3p", bufs=NT)
    vecs = tc.alloc_tile_pool(name="vecs", bufs=1)
    xtp = tc.alloc_tile_pool(name="xtp", bufs=1)
    xps = tc.alloc_tile_pool(name="xps", bufs=NT)
    xqp = tc.alloc_tile_pool(name="xqp", bufs=NT)

    # ones tiles via memset: pacer lhsT + softmax-sum matmuls, no DMA.
    ones32_sb = vecs.tile([128, 128], F32, tag="ones32")
    nc.vector.memset(ones32_sb[:, :], 1.0)
    ones128_sb = vecs.tile([128, 128], F32R, tag="ones128")
    nc.vector.tensor_copy(out=ones128_sb[:, :], in_=ones32_sb[:, :])
    onesb_sb = vecs.tile([128, 128], BF16, tag="onesb")
    nc.vector.tensor_copy(out=onesb_sb[:, :], in_=ones32_sb[:, :])
    # pacer moving tile (no DMA dep: pacing can start at t=0)
    pmov_sb = vecs.tile([128, CW], BF16, tag="pmov")
    nc.vector.memset(pmov_sb[:, :], 0.0)

    # ---- DMA in (sync-queue FIFO: tiny first, K2, x, W3T, xT) -------------
    vp_sb = vecs.tile([128, 24], F32, tag="vp")
    nc.sync.dma_start(out=vp_sb[:, :], in_=d["vp"])
    selT_sb = vecs.tile([GPT, 128], F32, tag="selT")
    nc.sync.dma_start(out=selT_sb[:, :], in_=d["selT"])

    gnw_sb = vp_sb[:, 0:NT]
    gnb_sb = vp_sb[:, NT:2 * NT]
    wkbq_sb = vp_sb[:, 2 * NT:3 * NT]
    wobv_sb = vp_sb[:, 3 * NT:4 * NT]
    sel_sb = vp_sb[:, 4 * NT:4 * NT + GPT]

    def load_w(pool, name, tag):
        tiles = []
        r = d[name].rearrange("(t p) m -> t p m", p=128)
        for t in range(NT):
            wt = pool.tile([128, C], BF16, tag=tag)
            nc.sync.dma_start(out=wt[:, :], in_=r[t])
            tiles.append(wt)
        return tiles

    k2_sb = load_w(k2p, "K2", "K2")    # [a_part, b] raw; A-scaled in place later

    x_sb = []
    x_t = d["x"].rearrange("(t p) n -> t p n", p=128)
    xsplits = []
    for t in range(NT):
        xt = xp.tile([128, N], BF16, tag="x", name=f"xt{t}")
        nsp = 2 if t < NT - 1 else 4   # last tile lands in quarters
        w = N // nsp
        for hh in range(nsp):
            nc.sync.dma_start(out=xt[:, hh * w:(hh + 1) * w],
                              in_=x_t[t][:, hh * w:(hh + 1) * w])
        xsplits.append([(hh * w, (hh + 1) * w) for hh in range(nsp)])
        x_sb.append(xt)

    w3_sb = load_w(w3p, "W3T", "W3T")  # [b_part, co] raw; A-scaled in place later

    # keys-major x^T from host, streamed in 4 pieces (first needed at loop
    # start; last needed ~60us in)
    xT_sb = xtp.tile([128, JT * C], BF16, tag="xT")
    for q in range(4):
        w = JT * C // 4
        nc.sync.dma_start(out=xT_sb[:, q * w:(q + 1) * w],
                          in_=d["xT"][:, q * w:(q + 1) * w])
    xTr = xT_sb.rearrange("p (j c) -> p j c", c=C)

    A_sb = vecs.tile([128, NT], F32, tag="A")
    B_sb = vecs.tile([128, NT], F32, tag="B")
    Bb_sb = vecs.tile([128, NT], BF16, tag="Bb")
    cbA_sb = vecs.tile([128, NT], F32, tag="cbA")
    wkbqA_sb = vecs.tile([128, NT], F32, tag="wkbqA")
    fb_sb = vecs.tile([128, NT], F32, tag="fb")

    # ---- GroupNorm stats -> per-channel affine A, B -----------------------
    with tc.tile_pool(name="stp", bufs=4) as stp, \
         tc.tile_pool(name="pace", bufs=2, space="PSUM") as pacep, \
         tc.tile_pool(name="pssm", bufs=2, space="PSUM") as ps_sm:
        npace = [0]

        def emit_pace(n, rhs=None):
            # 512-wide bf16 matmuls that keep the HAM clock gate warm; rhs
            # pins them behind a DMA landing (or none: free-running).
            for _ in range(n):
                wt = pacep.tile([128, CW], F32, tag="pace",
                                name=f"pc{npace[0]}")
                npace[0] += 1
                nc.tensor.matmul(out=wt[:, :], lhsT=onesb_sb[:, :],
                                 rhs=pmov_sb[:, :] if rhs is None else rhs,
                                 start=True, stop=True)

        # free-running pacers: spin the PE from t=0 through the framework
        # preamble + K2 DMA so the clock is hot when x starts landing.
        emit_pace(44)

        gps_t = []
        for t in range(NT):
            st = stp.tile([128, 8, 6], F32, tag="bnst", name=f"bnst{t}")
            xr = x_sb[t][:, :].rearrange("p (s n) -> p s n", s=8)
            for pi, (lo, hi) in enumerate(xsplits[t]):
                # pace pinned to this piece's landing
                emit_pace(6 if hi - lo == 2048 else 3,
                          rhs=x_sb[t][:, lo:lo + CW])
                for s in range(lo // 512, hi // 512):
                    nc.vector.bn_stats(out=st[:, s, :], in_=xr[:, s, :])
            mv = stp.tile([128, 2], F32, tag="mv", name=f"mv{t}")
            nc.vector.bn_aggr(out=mv[:, :], in_=st[:, :, :])
            st2 = stp.tile([128, 2], F32, tag="st2", name=f"st2_{t}")
            nc.vector.tensor_copy(out=st2[:, 0:1], in_=mv[:, 0:1])
            nc.vector.tensor_mul(out=st2[:, 1:2], in0=mv[:, 0:1], in1=mv[:, 0:1])
            nc.vector.tensor_add(out=st2[:, 1:2], in0=st2[:, 1:2], in1=mv[:, 1:2])
            gps = ps_sm.tile([GPT, 2], F32, tag="gps", name=f"gps{t}")
            nc.tensor.matmul(out=gps[:, :], lhsT=sel_sb, rhs=st2[:, :],
                             start=True, stop=True)
            gps_t.append(gps)
        # bridge pacers across the stats->A chain (DVE/ACT latency, PE idle)
        emit_pace(6)

        # group mean / rstd; all DVE preps first, then batched ACT sqrts
        grp_t = []
        for t in range(NT):
            grp = stp.tile([GPT, 2], F32, tag="grp", name=f"grp{t}")
            nc.vector.tensor_scalar_mul(out=grp[:, :], in0=gps_t[t][:, :], scalar1=GDIV)
            gtmp = stp.tile([GPT, 1], F32, tag="gtmp", name=f"gtmp{t}")
            nc.vector.tensor_mul(out=gtmp[:, :], in0=grp[:, 0:1], in1=grp[:, 0:1])
            nc.vector.tensor_sub(out=grp[:, 1:2], in0=grp[:, 1:2], in1=gtmp[:, :])
            nc.vector.tensor_scalar_add(out=grp[:, 1:2], in0=grp[:, 1:2], scalar1=EPS)
            grp_t.append(grp)
        for t in range(NT):
            nc.scalar.activation(out=grp_t[t][:, 1:2], in_=grp_t[t][:, 1:2],
                                 func=AF.Sqrt, bias=0.0, scale=1.0)
        # dummy exp: pulls the ~2.7us Exp ACT_TABLE_LOAD into the prologue
        scr = stp.tile([128, 1], F32, tag="scr")
        nc.scalar.activation(out=scr[:, :], in_=ones32_sb[:, 0:1], func=AF.Exp,
                             bias=0.0, scale=1.0)
        emit_pace(6)
        for t in range(NT):
            nc.vector.reciprocal(out=grp_t[t][:, 1:2], in_=grp_t[t][:, 1:2])
            mrp = ps_sm.tile([128, 2], F32, tag="gps", name=f"mrp{t}")
            nc.tensor.matmul(out=mrp[:, :], lhsT=selT_sb[:, :], rhs=grp_t[t][:, :],
                             start=True, stop=True)
            tcol = slice(t, t + 1)
            nc.vector.tensor_mul(out=A_sb[:, tcol], in0=gnw_sb[:, tcol], in1=mrp[:, 1:2])
            nc.vector.tensor_mul(out=B_sb[:, tcol], in0=mrp[:, 0:1], in1=A_sb[:, tcol])
            nc.vector.tensor_sub(out=B_sb[:, tcol], in0=gnb_sb[:, tcol], in1=B_sb[:, tcol])
        nc.vector.tensor_copy(out=Bb_sb[:, :], in_=B_sb[:, :])
        emit_pace(4)

    ps_mm = tc.alloc_tile_pool(name="psmm", bufs=3, space="PSUM")

    # ---- qk bias cb = K2^T B + wk^T bq (needs raw K2, so before scaling) --
    nc.vector.tensor_mul(out=wkbqA_sb[:, :], in0=A_sb[:, :], in1=wkbq_sb)
    for bb in range(NT):
        bps = ps_mm.tile([128, 1], F32, tag="mm", name=f"cb{bb}")
        for a in range(NT):
            nc.tensor.matmul(out=bps[:, :],
                             lhsT=k2_sb[a][:, bb * 128:(bb + 1) * 128],
                             rhs=Bb_sb[:, a:a + 1],
                             start=(a == 0), stop=(a == NT - 1))
        # cbA = A*(cb_psum) + A*wkbq
        nc.vector.tensor_scalar(out=cbA_sb[:, bb:bb + 1], in0=bps[:, :],
                                scalar1=A_sb[:, bb:bb + 1],
                                scalar2=wkbqA_sb[:, bb:bb + 1],
                                op0=ALU.mult, op1=ALU.add)

    # ---- K2A = A (.) K2 in place, then qk2 chunk 0 ------------------------
    for a in range(NT):
        nc.vector.tensor_scalar_mul(out=k2_sb[a][:, :], in0=k2_sb[a][:, :],
                                    scalar1=A_sb[:, a:a + 1])

    qkp = tc.alloc_tile_pool(name="qkp", bufs=NT)

    def emit_qk(ch):
        csl = slice(ch * CW, (ch + 1) * CW)
        qk2 = []
        for bb in range(NT):
            qps = ps_mm.tile([128, CW], F32, tag="mm")
            for a in range(NT):
                nc.tensor.matmul(out=qps[:, :],
                                 lhsT=k2_sb[a][:, bb * 128:(bb + 1) * 128],
                                 rhs=x_sb[a][:, csl],
                                 start=(a == 0), stop=(a == NT - 1))
            qk = qkp.tile([128, CW], BF16, tag="qk")
            nc.vector.tensor_scalar(out=qk[:, :], in0=qps[:, :],
                                    scalar1=A_sb[:, bb:bb + 1],
                                    scalar2=cbA_sb[:, bb:bb + 1],
                                    op0=ALU.mult, op1=ALU.add)
            qk2.append(qk)
        return qk2

    qk2_ch = emit_qk(0)

    # ---- out bias fb = W3T^T B + (wo@bv + bo) (raw W3T, before scaling) ---
    for cob in range(NT):
        fps = ps_mm.tile([128, 1], F32, tag="mm", name=f"fb{cob}")
        for b in range(NT):
            nc.tensor.matmul(out=fps[:, :],
                             lhsT=w3_sb[b][:, cob * 128:(cob + 1) * 128],
                             rhs=Bb_sb[:, b:b + 1],
                             start=(b == 0), stop=(b == NT - 1))
        nc.vector.tensor_add(out=fb_sb[:, cob:cob + 1], in0=fps[:, :],
                             in1=wobv_sb[:, cob:cob + 1])

    # ---- W3AT = A (.) W3T in place ----------------------------------------
    for b in range(NT):
        nc.vector.tensor_scalar_mul(out=w3_sb[b][:, :], in0=w3_sb[b][:, :],
                                    scalar1=A_sb[:, b:b + 1])

    # xq = x[:, 0:NQ] + fb (DVE; emitted after the qk2 affine so it doesn't
    # delay the first logits matmul -- runs in the main loop's DVE shadow)
    xq_sb = []
    for co in range(NT):
        xq = xqp.tile([128, NQ], F32, tag="xq", name=f"xq{co}")
        for h in range(NCH):
            sl = slice(h * CW, (h + 1) * CW)
            nc.vector.tensor_scalar_add(out=xq[:, sl], in0=x_sb[co][:, sl],
                                        scalar1=fb_sb[:, co:co + 1])
        xq_sb.append(xq)

    # ---- attention chunks -------------------------------------------------
    ps_o = tc.alloc_tile_pool(name="pso", bufs=NT, space="PSUM")
    pp = tc.alloc_tile_pool(name="pp", bufs=6)
    outp = tc.alloc_tile_pool(name="outp", bufs=2)
    smsb = tc.alloc_tile_pool(name="smsb", bufs=2)

    qk2_next = qk2_ch

    for ch in range(NCH):
        csl = slice(ch * CW, (ch + 1) * CW)
        qk2_ch = qk2_next

        o_ps = [ps_o.tile([128, CW], F32, tag="o", name=f"o{ch}_{i}") for i in range(4)]
        sacc = smsb.tile([128, CW], F32R, tag="sacc", name=f"sacc{ch}")
        P_t = [None] * JT
        for jt in range(JT):
            jsl = slice(jt * 128, (jt + 1) * 128)
            lps = ps_mm.tile([128, CW], F32, tag="mm")
            for b in range(NT):
                nc.tensor.matmul(out=lps[:, :], lhsT=x_sb[b][:, jsl],
                                 rhs=qk2_ch[b][:, :],
                                 start=(b == 0), stop=(b == NT - 1))
            P = pp.tile([128, CW], BF16, tag="P")
            nc.scalar.activation(out=P[:, :], in_=lps[:, :], func=AF.Exp,
                                 bias=0.0, scale=SCALE)
            P_t[jt] = P
            # xP = x @ P accumulation lags one iteration: P[jt-1] is ready
            if jt > 0:
                for b in range(4):
                    nc.tensor.matmul(out=o_ps[b][:, :],
                                     lhsT=xTr[:, jt - 1, b * 128:(b + 1) * 128],
                                     rhs=P_t[jt - 1][:, :],
                                     start=(jt == 1), stop=False,
                                     skip_group_check=True)
            # running softmax denominator on DVE (jt<=30; P31 via matmul)
            if jt == 0:
                nc.vector.tensor_copy(out=sacc[:, :], in_=P[:, :])
            elif jt < JT - 1:
                nc.vector.tensor_add(out=sacc[:, :], in0=_f32(sacc[:, :]),
                                     in1=P[:, :])

        # 1/s: s = ones@sacc + ones@P31, ready before the epilogue needs it;
        # each x@P bank drains to SBUF as its last matmul stops (2 on ACT,
        # 2 on DVE to halve the serial drain latency)
        xP_sb = []
        for b in range(4):
            nc.tensor.matmul(out=o_ps[b][:, :],
                             lhsT=xTr[:, JT - 1, b * 128:(b + 1) * 128],
                             rhs=P_t[JT - 1][:, :],
                             start=False, stop=True, skip_group_check=True)
            xs = xps.tile([128, CW], BF16, tag="xps", name=f"xps{ch}_{b}")
            if b % 2 == 0:
                nc.scalar.activation(out=xs[:, :], in_=o_ps[b][:, :],
                                     func=AF.Copy, bias=0.0, scale=1.0)
            else:
                nc.vector.tensor_copy(out=xs[:, :], in_=o_ps[b][:, :])
            xP_sb.append(xs)
        rbp = ps_mm.tile([128, CW], F32, tag="mm")
        nc.tensor.matmul(out=rbp[:, :], lhsT=ones128_sb[:, :], rhs=sacc[:, :],
                         start=True, stop=False)
        nc.tensor.matmul(out=rbp[:, :], lhsT=onesb_sb[:, :],
                         rhs=P_t[JT - 1][:, :], start=False, stop=True)
        rsb = smsb.tile([128, CW], F32, tag="rsb")
        nc.vector.reciprocal_approx_fast(out=rsb[:, :], in_=rbp[:, :])
        if ch + 1 < NCH:
            qk2_next = emit_qk(ch + 1)

        # o = W3A @ xP (16 small matmuls), normalize, +x+fb, DMA out
        for co in range(4):
            ops = ps_mm.tile([128, CW], F32, tag="mm")
            for b in range(4):
                nc.tensor.matmul(out=ops[:, :],
                                 lhsT=w3_sb[b][:, co * 128:(co + 1) * 128],
                                 rhs=xP_sb[b][:, :],
                                 start=(b == 0), stop=(b == 3))
            ot_ = outp.tile([128, CW], F32, tag="osb", name=f"n{ch}_{co}")
            nc.vector.tensor_mul(out=ot_[:, :], in0=ops[:, :], in1=rsb[:, :])
            ou = outp.tile([128, CW], F32, tag="oadd", name=f"r{ch}_{co}")
            nc.vector.tensor_add(out=ou[:, :], in0=ot_[:, :],
                                 in1=xq_sb[co][:, csl])
            nc.sync.dma_start(out=d["out"][co * 128:(co + 1) * 128, csl], in_=ou[:, :])

    for p in (smsb, outp, pp, ps_o, qkp, ps_mm, xqp, xps, xtp, vecs, w3p, k2p, xp):
        p.release()


def _sel_consts():
    sel = np.zeros((128, GPT), np.float32)
    for p in range(128):
        sel[p, p // 16] = 1.0
    return sel, np.ascontiguousarray(sel.T)


def kernel(x, gn_w, gn_b, wq, bq, wk, bk, wv, bv, wo, bo):
    del bk  # exactly cancelled by softmax shift invariance
    if "nc" not in _CACHE:
        _CACHE["nc"] = _build_bass()
    nc = _CACHE["nc"]
    bf16 = ml_dtypes.bfloat16

    x = np.ascontiguousarray(np.asarray(x, np.float32)).reshape(B, C, N)
    wq64 = np.asarray(wq, np.float64)
    wk64 = np.asarray(wk, np.float64)
    wv64 = np.asarray(wv, np.float64)
    wo64 = np.asarray(wo, np.float64)
    K2 = np.ascontiguousarray((wq64.T @ wk64).astype(bf16))
    W3T = np.ascontiguousarray((wo64 @ wv64).T.astype(bf16))
    wkbq = (wk64.T @ np.asarray(bq, np.float64)).astype(np.float32)
    wobvbo = (wo64 @ np.asarray(bv, np.float64)
              + np.asarray(bo, np.float64)).astype(np.float32)
    sel, selT = _sel_consts()
    vp = np.concatenate([
        np.asarray(gn_w, np.float32).reshape(NT, 128).T,
        np.asarray(gn_b, np.float32).reshape(NT, 128).T,
        wkbq.reshape(NT, 128).T,
        wobvbo.reshape(NT, 128).T,
        sel,
    ], axis=1)
    vp = np.ascontiguousarray(vp)

    in_maps = []
    for core in range(8):
        b, qb = core // 4, core % 4
        xb = np.roll(x[b], -qb * NQ, axis=1)
        xb_bf = np.ascontiguousarray(xb.astype(bf16))
        # keys-major x^T in the device tile layout: [p, jt*C + c] = x[c, jt*128+p]
        xT_bf = np.ascontiguousarray(
            xb_bf.T.reshape(JT, 128, C).transpose(1, 0, 2).reshape(128, JT * C))
        in_maps.append({"x": xb_bf, "xT": xT_bf, "K2": K2, "W3T": W3T,
                        "vp": vp, "selT": selT})

    _CACHE["last_in_maps"] = in_maps
    res = run_bass_kernel_spmd(nc, in_maps, list(range(8))).results
    out = np.empty((B, C, N), np.float32)
    for core in range(8):
        b, qb = core // 4, core % 4
        out[b][:, qb * NQ:(qb + 1) * NQ] = res[core]["out"]
    return out.reshape(B, C, HH, WW)
